# revision 34
# baseline (speedup 1.0000x reference)
"""BinaryAttention on 8 TRN2 NeuronCores (Bass/Tile, SPMD).

Math (per reference):
  Wb = alpha * sign(W), alpha[o] = mean_c |W[o,c]|
  q/k/v = x @ Wb_{q,k,v}^T + b;   att = softmax(q k^T / sqrt(Dh));
  y = att @ v;  out = y @ Wb_p^T + bp

Sharding (8 cores = 4 batch groups x 2 cores): core c handles batch c//2
with heads [8j, 8j+8) for j = c%2 (head-tensor-parallel within the pair).
After attention, a pairwise AllGather assembles y [1024, T_tile] per pair;
proj is output-column sharded (core j computes out cols [512j, 512j+512)).

Precision plan (validated vs reference in fp64/numpy, rel ~1.4e-2 < 2e-2):
  - q,k matmuls: fp8(e4m3) x and sign-weights, DoubleRow perf mode (2x);
    alpha/bias applied fp32 -> q,k in bf16.
  - scores: bf16, two PE row-tiles (heads at partitions 0-63 / 64-127).
  - exp: Scalar engine exact exp -> fp8 att for 3/4 of s-chunks; DVE
    computes a bit-trick fast exp (int8 = s*A + B bitcast as e4m3) for 1/4.
  - att@v: fp8 DoubleRow (2x); v kept unscaled (alpha_v/bias_v folded into
    the normalization: y = alpha_v*(ym/den) + bias_v).
  - v matmul: bf16 "swapped" form (stationary x-chunks, moving sign-cols)
    which yields v in [s, dims] layout directly -- no PE transposes.
  - proj: bf16 (fp8 y would push error past tolerance).
"""

import numpy as np
import ml_dtypes

import concourse.bass as bass
import concourse.bacc as bacc
import concourse.tile as tile
from concourse import mybir
from concourse.bass_utils import run_bass_kernel_spmd

NC = 8
B, T, C = 4, 2048, 1024
H, DH = 16, 64
HPC = 8          # heads per core
NHP = 4          # head-pairs per core
OS = 512         # per-core o-slice (8 heads * 64 = 512 dims)
KC = C // 128    # contraction chunks
NT = 512         # t-tile (one psum bank of fp32)
NSC = T // 128   # s-chunks (16)
SCALE = DH ** -0.5
LOG2E = 1.4426950408889634
# DVE fast-exp: e4m3 bits = round(s*scale*log2e*8 + 56 + C8)
A8 = SCALE * LOG2E * 8.0
B8 = 56.0 - 0.5
DVE_EVERY = 3    # every 3rd s-chunk's exp goes to DVE

F32 = mybir.dt.float32
BF16 = mybir.dt.bfloat16
F8 = mybir.dt.float8e4
I8 = mybir.dt.int8
DR = mybir.MatmulPerfMode.DoubleRow

_CACHED = {}


def _build():
    nc = bacc.Bacc("TRN2", target_bir_lowering=False, debug=False, num_devices=NC)

    xT8 = nc.dram_tensor("xT8", [C, T], F8, kind="ExternalInput")
    xTb = nc.dram_tensor("xTb", [C, T], BF16, kind="ExternalInput")
    sq8 = nc.dram_tensor("sq8", [C, OS], F8, kind="ExternalInput")
    sk8 = nc.dram_tensor("sk8", [C, OS], F8, kind="ExternalInput")
    svb = nc.dram_tensor("svb", [C, OS], BF16, kind="ExternalInput")
    spb = nc.dram_tensor("spb", [C, OS], BF16, kind="ExternalInput")
    scl_d = nc.dram_tensor("scl", [OS, 6], F32, kind="ExternalInput")
    svv_d = nc.dram_tensor("svv", [DH, 2 * HPC], F32, kind="ExternalInput")
    out_t = nc.dram_tensor("out_t", [OS, T], F32, kind="ExternalOutput")

    x8r = xT8.rearrange("(k p) n -> p k n", p=128)
    xbr = xTb.rearrange("(k p) n -> p k n", p=128)

    with tile.TileContext(nc, num_cores=NC) as tc:
        with (
            tc.tile_pool(name="const", bufs=1) as const,
            tc.tile_pool(name="attp", bufs=8) as attp,
            tc.tile_pool(name="xbpool", bufs=12) as xbpool,
            tc.tile_pool(name="ypool", bufs=6) as ypool,
            tc.tile_pool(name="ygpool", bufs=10) as ygpool,
            tc.tile_pool(name="outp", bufs=4) as outp,
            tc.tile_pool(name="sc_ps", bufs=3, space="PSUM") as sc_ps,
            tc.tile_pool(name="y_ps", bufs=2, space="PSUM") as y_ps,
            tc.tile_pool(name="dram", bufs=1, space="DRAM") as dram,
        ):
            # ---------------- prologue: weights / x / scalars ----------
            sq_sb = const.tile([128, KC, OS], F8, tag="sq")
            sk_sb = const.tile([128, KC, OS], F8, tag="sk")
            sq8r = sq8.rearrange("(k p) o -> p k o", p=128)
            sk8r = sk8.rearrange("(k p) o -> p k o", p=128)
            for k4 in range(0, KC, 4):
                nc.sync.dma_start(sq_sb[:, k4:k4 + 4, :], sq8r[:, k4:k4 + 4, :])
                nc.sync.dma_start(sk_sb[:, k4:k4 + 4, :], sk8r[:, k4:k4 + 4, :])
            scl_sb = const.tile([128, NHP, 6], F32, tag="scl")
            nc.sync.dma_start(scl_sb[:], scl_d.rearrange("(c p) o -> p c o", p=128))
            aq_sb = scl_sb[:, :, 0:1]
            ak_sb = scl_sb[:, :, 1:2]
            bq_sb = scl_sb[:, :, 2:3]
            bk_sb = scl_sb[:, :, 3:4]
            ap_sb = scl_sb[:, :, 4:5]
            bp_sb = scl_sb[:, :, 5:6]
            svv_sb = const.tile([DH, 2 * HPC], F32, tag="svv")
            nc.sync.dma_start(svv_sb[:], svv_d[:])
            av_sb = svv_sb[:, 0:HPC]
            bv_sb = svv_sb[:, HPC:2 * HPC]
            x8_sb = const.tile([128, KC, T], F8, tag="x8")
            for k2 in range(0, KC, 2):
                nc.sync.dma_start(x8_sb[:, k2:k2 + 2, :], x8r[:, k2:k2 + 2, :])
            sv_sb = const.tile([128, KC, OS], BF16, tag="sv")
            svbr = svb.rearrange("(k p) o -> p k o", p=128)
            for k4 in range(0, KC, 4):
                nc.sync.dma_start(sv_sb[:, k4:k4 + 4, :], svbr[:, k4:k4 + 4, :])
            # xb streamed per s-chunk (v matmul stationary): [128, KC, 128]
            sp_sb = const.tile([128, KC, OS], BF16, tag="sp")
            spbr = spb.rearrange("(k p) o -> p k o", p=128)
            for k4 in range(0, KC, 4):
                nc.sync.dma_start(sp_sb[:, k4:k4 + 4, :], spbr[:, k4:k4 + 4, :])

            # q,k per head-pair in bf16 [128 dims, T]; v in fp8
            # [s-part, scp, pair, head, DH+1] with a ones column for denoms.
            q_sb = const.tile([128, NHP, T], BF16, tag="qsb")
            k_sb = const.tile([128, NHP, T], BF16, tag="ksb")
            # inner dim padded to 66 so the DoubleRow pair step (8*66=528B)
            # meets the dual-fp8 ldweights 16B stride alignment
            v_sb = const.tile([128, NSC // 2, 2, HPC, DH + 2], F8, tag="vsb")
            nc.vector.memset(v_sb[:, :, :, :, DH:DH + 1], 1.0)

            y_gath = {}
            LT = T // NT - 1
            for tt in range(T // NT):
                for half in range(2):
                    if tt == LT and half == 0:
                        continue
                    yb = dram.tile([OS // 2, NT], BF16, tag=f"ybnc{tt}{half}")
                    yg = dram.tile([C // 2, NT], BF16, tag=f"ygth{tt}{half}")
                    y_gath[(tt, half)] = (yb, yg)
            for q in ("0a", "0b"):
                yb = dram.tile([OS // 4, NT], BF16, tag=f"ybnc{LT}{q}")
                yg = dram.tile([C // 4, NT], BF16, tag=f"ygth{LT}{q}")
                y_gath[(LT, q)] = (yb, yg)

            # ---------------- QKV ---------------------------------------
            def qk_chunk(wn, hp, nt):
                s_sb, a_sb, b_sb, dst = {
                    "q": (sq_sb, aq_sb, bq_sb, q_sb),
                    "k": (sk_sb, ak_sb, bk_sb, k_sb),
                }[wn]
                psw = sc_ps.tile([128, 2, NT], F32, name=f"ps{wn}{hp}{nt}", tag="sps")
                ps = psw[:, 0, :]
                for j in range(KC // 2):
                    nc.tensor.matmul(
                        ps,
                        s_sb[:, 2 * j:2 * j + 2, hp * 128:(hp + 1) * 128],
                        x8_sb[:, 2 * j:2 * j + 2, nt * NT:(nt + 1) * NT],
                        start=(j == 0), stop=(j == KC // 2 - 1),
                        perf_mode=DR,
                    )
                nc.vector.tensor_scalar(
                    out=dst[:, hp, nt * NT:(nt + 1) * NT], in0=ps,
                    scalar1=a_sb[:, hp, :], scalar2=b_sb[:, hp, :],
                    op0=mybir.AluOpType.mult, op1=mybir.AluOpType.add,
                )

            def v_xload(sc):
                xbt = xbpool.tile([128, KC, 128], BF16, name=f"xb{sc}", tag="xbs")
                nc.sync.dma_start(xbt[:], xbr[:, :, sc * 128:(sc + 1) * 128])
                return xbt

            def v_chunk(sc, xbt):
                # swapped: stationary x bf16 chunk [128c, 128s],
                # moving sign cols [128c, 512 dims] -> psum [128 s, 512 d]
                psw = sc_ps.tile([128, 2, NT], F32, name=f"psv{sc}", tag="sps")
                ps = psw[:, 0, :]
                for kc in range(KC):
                    nc.tensor.matmul(
                        ps,
                        xbt[:, kc, :],
                        sv_sb[:, kc, :],
                        start=(kc == 0), stop=(kc == KC - 1),
                    )
                nc.vector.tensor_copy(
                    out=v_sb[:, sc // 2, sc % 2, :, 0:DH],
                    in_=ps.rearrange("p (h d) -> p h d", h=HPC),
                )

            # ---------------- attention ---------------------------------
            pend_norm = []

            def norm_stage1(hp, tt, h, yc):
                # issue the denominator reciprocal round-trips early so the
                # DRAM latency overlaps the next unit's attention
                hg = hp * 2 + h
                r_d = dram.tile([1, NT], F32, tag=f"rd{tt}{hg}")
                nc.sync.dma_start(r_d[:], yc[DH:DH + 1, :])
                rf = ypool.tile([DH, NT // DH], F32, tag="rf")
                nc.sync.dma_start(
                    rf[:], r_d.rearrange("one (p f) -> (one p) f", p=DH))
                rfi = ypool.tile([DH, NT // DH], F32, tag="rfi")
                nc.vector.reciprocal(rfi[:], rf[:])
                ri_d = dram.tile([DH, NT // DH], F32, tag=f"rid{tt}{hg}")
                nc.sync.dma_start(ri_d[:], rfi[:])
                rbi = ypool.tile([DH, NT], F32, tag="rbi")
                nc.sync.dma_start(
                    rbi[:],
                    bass.AP(tensor=ri_d.tensor, offset=ri_d.offset,
                            ap=[[0, DH], [1, NT]]),
                )
                return rbi

            def norm_stage2(item):
                hp, tt, h, yc, rbi = item
                hg = hp * 2 + h
                yt = ypool.tile([DH, NT], F32, tag="yt")
                nc.gpsimd.tensor_mul(yt[:], yc[0:DH, :], rbi[:])
                yb_out = ypool.tile([DH, NT], BF16, tag="ybf")
                nc.gpsimd.tensor_scalar(
                    out=yb_out[:], in0=yt[:],
                    scalar1=av_sb[:, hg:hg + 1], scalar2=bv_sb[:, hg:hg + 1],
                    op0=mybir.AluOpType.mult, op1=mybir.AluOpType.add,
                )
                half, row = divmod(hg * DH, OS // 2)
                if tt == LT and half == 0:
                    half = "0a" if row < OS // 4 else "0b"
                    row = row % (OS // 4)
                nc.sync.dma_start(
                    y_gath[(tt, half)][0][row:row + DH, :], yb_out[:])

            def attention_unit(hp, tt, fills=()):
                fills = list(fills)
                t0 = tt * NT
                att_tiles = []
                for scp in range(NSC // 2):
                    at = attp.tile([128, 2, 2, NT], F8,
                                   name=f"at{hp}{tt}{scp}", tag="att")
                    att_tiles.append(at)
                for sc in range(NSC):
                    s0 = sc * 128
                    pss = sc_ps.tile([128, 2, NT], F32,
                                     name=f"s{hp}{tt}{sc}", tag="sps")
                    nc.tensor.matmul(
                        pss[:, 0, :], k_sb[0:DH, hp, s0:s0 + 128],
                        q_sb[0:DH, hp, t0:t0 + NT], start=True, stop=True,
                    )
                    nc.tensor.matmul(
                        pss[:, 1, :], k_sb[DH:128, hp, s0:s0 + 128],
                        q_sb[DH:128, hp, t0:t0 + NT], start=True, stop=True,
                    )
                    at = att_tiles[sc // 2]
                    if sc % DVE_EVERY == DVE_EVERY - 1:
                        nc.vector.tensor_scalar(
                            out=at[:, sc % 2, :, :].bitcast(I8), in0=pss[:],
                            scalar1=A8, scalar2=B8,
                            op0=mybir.AluOpType.mult, op1=mybir.AluOpType.add,
                        )
                    else:
                        nc.scalar.activation(
                            out=at[:, sc % 2, :, :], in_=pss[:],
                            func=mybir.ActivationFunctionType.Exp, scale=SCALE,
                        )
                    if sc % 4 == 3 and fills:
                        fills.pop(0)()
                while fills:
                    fills.pop(0)()
                psA = y_ps.tile([DH + 1, NT], F32, name=f"yA{hp}{tt}", tag="yps")
                psB = y_ps.tile([DH + 1, NT], F32, name=f"yB{hp}{tt}", tag="yps")
                for scp in range(NSC // 2):
                    at = att_tiles[scp]
                    for h, psy in ((0, psA), (1, psB)):
                        nc.tensor.matmul(
                            psy[:],
                            v_sb[:, scp, :, hp * 2 + h, 0:DH + 1],
                            at[:, :, h, :],
                            start=(scp == 0), stop=(scp == NSC // 2 - 1),
                            perf_mode=DR,
                        )
                for h, psy in ((0, psA), (1, psB)):
                    yc = ypool.tile([DH + 1, NT], F32,
                                    name=f"yc{hp}{tt}{h}", tag="yc")
                    nc.vector.tensor_copy(yc[:], psy[:])
                    rbi = norm_stage1(hp, tt, h, yc)
                    pend_norm.append((hp, tt, h, yc, rbi))
                while len(pend_norm) > 2:
                    norm_stage2(pend_norm.pop(0))

            def flush_norms():
                while pend_norm:
                    norm_stage2(pend_norm.pop(0))

            def gather_half(tt, half):
                yb, yg = y_gath[(tt, half)]
                nc.gpsimd.collective_compute(
                    "AllGather", mybir.AluOpType.bypass,
                    replica_groups=[[0, 1], [2, 3], [4, 5], [6, 7]],
                    ins=[yb.opt()], outs=[yg.opt()],
                )

            def _yg_load(tt, g):
                # gathered half h holds full-y rows [0:256]+[512:768] (h=0)
                # or [256:512]+[768:1024] (h=1)
                half, row = divmod((g % 4) * 128, OS // 2)
                if tt == LT and half == 0:
                    half = "0a" if row == 0 else "0b"
                    row = (g // 4) * (OS // 4)
                else:
                    row = row + (g // 4) * (OS // 2)
                src_t = y_gath[(tt, half)][1]
                yg_sb = ygpool.tile([128, NT], BF16, name=f"yg{tt}{g}", tag="ygp")
                nc.gpsimd.dma_start(yg_sb[:], src_t[row:row + 128, :])
                return yg_sb

            # g-chunks living in gather-half 0 vs half 1; chains consume the
            # half-1 chunks first (half-1 gathers mid-tt, half-0 at tt end)
            G_H0, G_H1 = (0, 1, 4, 5), (2, 3, 6, 7)
            G_ORDER = list(G_H1) + list(G_H0)

            def proj_oc(tt, oc, ygs, pps=None):
                for _ in (0,):
                    if pps is not None and oc in pps:
                        pp = pps[oc]
                        gseq = [0, 4, 1, 5]
                        cont = True
                    else:
                        ppw = sc_ps.tile([128, 2, NT], F32,
                                         name=f"pp{tt}{oc}", tag="sps")
                        pp = ppw[:, 0, :]
                        gseq = list(G_ORDER)
                        cont = False
                    for i, g in enumerate(gseq):
                        nc.tensor.matmul(
                            pp, sp_sb[:, g, oc * 128:(oc + 1) * 128],
                            ygs[g][:], start=(not cont and i == 0),
                            stop=(i == len(gseq) - 1),
                        )
                    o_sb = outp.tile([128, NT], F32, name=f"o{tt}{oc}", tag="osb")
                    nc.vector.tensor_scalar(
                        out=o_sb[:], in0=pp,
                        scalar1=ap_sb[:, oc, :], scalar2=bp_sb[:, oc, :],
                        op0=mybir.AluOpType.mult, op1=mybir.AluOpType.add,
                    )
                    for pq in range(4):
                        nc.sync.dma_start(
                            out_t[oc * 128 + pq * 32:oc * 128 + (pq + 1) * 32,
                                  tt * NT:(tt + 1) * NT],
                            o_sb[pq * 32:(pq + 1) * 32, :])

            def proj_fills(tt, pps=None):
                ygs = {g: _yg_load(tt, g) for g in range(KC)}
                return [
                    (lambda oc=oc: proj_oc(tt, oc, ygs, pps))
                    for oc in range(NHP)
                ]

            def proj_first_half_fills(tt, pps):
                # fills that accumulate half-1 g-chunks for the first two
                # out-chunks in persistent psum tiles; the epilogue finishes
                # them with the half-0 chunks once the final gather lands
                ygs = {g: _yg_load(tt, g) for g in G_H1}
                pps["ygs_pre"] = ygs

                def one(oc):
                    ppw = sc_ps.tile([128, 2, NT], F32,
                                     name=f"pph{tt}{oc}", tag="sps")
                    pp = ppw[:, 0, :]
                    pps[oc] = pp
                    for i, g in enumerate(G_H1):
                        nc.tensor.matmul(
                            pp, sp_sb[:, g, oc * 128:(oc + 1) * 128],
                            ygs[g][:], start=(i == 0), stop=False,
                        )
                return [lambda oc=oc: one(oc) for oc in range(2)]

            # ---------------- schedule ----------------------------------
            for hp in range(NHP):
                for nt in range(T // NT):
                    qk_chunk("k", hp, nt)
            for hp in range(NHP):
                qk_chunk("q", hp, 0)
            v_xts = [v_xload(sc) for sc in range(NSC)]
            for sc in range(NSC // 2):
                v_chunk(sc, v_xts[sc])
            # second half of v and the remaining q chunks are emitted as
            # fills inside tt0's units so their PE work overlaps tt0's exp
            pend_fills = [
                (lambda sc=sc: v_chunk(sc, v_xts[sc]))
                for sc in range(NSC // 2, NSC)
            ] + [
                (lambda hp=hp, nt=nt: qk_chunk("q", hp, nt))
                for nt in range(1, T // NT) for hp in range(NHP)
            ]

            # the lag-2 pend_norm queue means: by the end of unit(hp, tt),
            # all norms of units two back are emitted -- so gather halves can
            # fire without explicit flushes (except the very last one).
            pps_last = {}
            HP_ORDER = (2, 3, 0, 1)
            for tt in range(T // NT):
                for ui, hp in enumerate(HP_ORDER):
                    if tt == 0:
                        nfill = 8 if ui == 0 else 4
                    elif tt == LT and ui == 3:
                        nfill = 4
                    else:
                        nfill = 2
                    take, pend_fills = pend_fills[:nfill], pend_fills[nfill:]
                    if tt == LT and ui == 3:
                        take = take + proj_first_half_fills(LT, pps_last)
                    attention_unit(hp, tt, take)
                    if ui == 2 and tt >= 1:
                        pend_fills = pend_fills + proj_fills(tt - 1)
                    if ui == 2:
                        gather_half(tt, 1)
                    if tt == LT and ui == 3:
                        gather_half(LT, "0a")
                flush_norms()
                gather_half(tt, 0 if tt < LT else "0b")
            for f in pend_fills:
                f()
            ygs_all = pps_last.pop("ygs_pre")
            ygs_all.update({g: _yg_load(LT, g) for g in G_H0})
            for oc in (2, 3, 0, 1):
                proj_oc(LT, oc, ygs_all, pps_last)

    nc.finalize()
    return nc


def _host_prep(x, Wq, bq, Wk, bk, Wv, bv, Wp, bp):
    F8N = ml_dtypes.float8_e4m3
    BF = ml_dtypes.bfloat16
    in_maps = []
    xt_b = [np.ascontiguousarray(x[b].T) for b in range(B)]
    for c in range(NC):
        b, j = c // 2, c % 2
        hs = slice(OS * j, OS * (j + 1))
        m = {
            "xT8": xt_b[b].astype(F8N),
            "xTb": xt_b[b].astype(BF),
            "sq8": np.ascontiguousarray(np.sign(Wq[hs]).T).astype(F8N),
            "sk8": np.ascontiguousarray(np.sign(Wk[hs]).T).astype(F8N),
            "svb": np.ascontiguousarray(np.sign(Wv[hs]).T).astype(BF),
            "spb": np.ascontiguousarray(np.sign(Wp[hs]).T).astype(BF),
            "scl": np.stack([
                np.abs(Wq[hs]).mean(1, dtype=np.float64).astype(np.float32),
                np.abs(Wk[hs]).mean(1, dtype=np.float64).astype(np.float32),
                bq[hs], bk[hs],
                np.abs(Wp[hs]).mean(1, dtype=np.float64).astype(np.float32),
                bp[hs]], axis=1),
            "svv": np.concatenate([
                np.abs(Wv[hs]).mean(1, dtype=np.float64)
                .reshape(HPC, DH).T.astype(np.float32),
                bv[hs].reshape(HPC, DH).T.astype(np.float32)], axis=1),
        }
        in_maps.append(m)
    return in_maps


def kernel(x, Wq, bq, Wk, bk, Wv, bv, Wp, bp, _trace=False, _trace_cores=None):
    if "nc" not in _CACHED:
        _CACHED["nc"] = _build()
    nc = _CACHED["nc"]
    in_maps = _host_prep(x, Wq, bq, Wk, bk, Wv, bv, Wp, bp)
    res = run_bass_kernel_spmd(
        nc, in_maps, core_ids=list(range(NC)),
        trace=_trace, trace_cores=_trace_cores,
    )
    _CACHED["last_results"] = res
    out = np.empty((B, T, C), dtype=np.float32)
    for b in range(B):
        full = np.concatenate(
            [res.results[2 * b]["out_t"], res.results[2 * b + 1]["out_t"]],
            axis=0)                     # [1024 o, 2048 t]
        out[b] = full.T
    return out


# revision 35
# speedup vs baseline: 1.0053x; 1.0053x over previous
"""BinaryAttention on 8 TRN2 NeuronCores (Bass/Tile, SPMD).

Math (per reference):
  Wb = alpha * sign(W), alpha[o] = mean_c |W[o,c]|
  q/k/v = x @ Wb_{q,k,v}^T + b;   att = softmax(q k^T / sqrt(Dh));
  y = att @ v;  out = y @ Wb_p^T + bp

Sharding (8 cores = 4 batch groups x 2 cores): core c handles batch c//2
with heads [8j, 8j+8) for j = c%2 (head-tensor-parallel within the pair).
After attention, a pairwise AllGather assembles y [1024, T_tile] per pair;
proj is output-column sharded (core j computes out cols [512j, 512j+512)).

Precision plan (validated vs reference in fp64/numpy, rel ~1.4e-2 < 2e-2):
  - q,k matmuls: fp8(e4m3) x and sign-weights, DoubleRow perf mode (2x);
    alpha/bias applied fp32 -> q,k in bf16.
  - scores: bf16, two PE row-tiles (heads at partitions 0-63 / 64-127).
  - exp: Scalar engine exact exp -> fp8 att for 3/4 of s-chunks; DVE
    computes a bit-trick fast exp (int8 = s*A + B bitcast as e4m3) for 1/4.
  - att@v: fp8 DoubleRow (2x); v kept unscaled (alpha_v/bias_v folded into
    the normalization: y = alpha_v*(ym/den) + bias_v).
  - v matmul: bf16 "swapped" form (stationary x-chunks, moving sign-cols)
    which yields v in [s, dims] layout directly -- no PE transposes.
  - proj: bf16 (fp8 y would push error past tolerance).
"""

import numpy as np
import ml_dtypes

import concourse.bass as bass
import concourse.bacc as bacc
import concourse.tile as tile
from concourse import mybir
from concourse.bass_utils import run_bass_kernel_spmd

NC = 8
B, T, C = 4, 2048, 1024
H, DH = 16, 64
HPC = 8          # heads per core
NHP = 4          # head-pairs per core
OS = 512         # per-core o-slice (8 heads * 64 = 512 dims)
KC = C // 128    # contraction chunks
NT = 512         # t-tile (one psum bank of fp32)
NSC = T // 128   # s-chunks (16)
SCALE = DH ** -0.5
LOG2E = 1.4426950408889634
# DVE fast-exp: e4m3 bits = round(s*scale*log2e*8 + 56 + C8)
A8 = SCALE * LOG2E * 8.0
B8 = 56.0 - 0.5
DVE_EVERY = 3    # every 3rd s-chunk's exp goes to DVE

F32 = mybir.dt.float32
BF16 = mybir.dt.bfloat16
F8 = mybir.dt.float8e4
I8 = mybir.dt.int8
DR = mybir.MatmulPerfMode.DoubleRow

_CACHED = {}


def _build():
    nc = bacc.Bacc("TRN2", target_bir_lowering=False, debug=False, num_devices=NC)

    xT8 = nc.dram_tensor("xT8", [C, T], F8, kind="ExternalInput")
    xTb = nc.dram_tensor("xTb", [C, T], BF16, kind="ExternalInput")
    sq8 = nc.dram_tensor("sq8", [C, OS], F8, kind="ExternalInput")
    sk8 = nc.dram_tensor("sk8", [C, OS], F8, kind="ExternalInput")
    svb = nc.dram_tensor("svb", [C, OS], BF16, kind="ExternalInput")
    spb = nc.dram_tensor("spb", [C, OS], BF16, kind="ExternalInput")
    scl_d = nc.dram_tensor("scl", [OS, 6], F32, kind="ExternalInput")
    svv_d = nc.dram_tensor("svv", [DH, 2 * HPC], F32, kind="ExternalInput")
    out_t = nc.dram_tensor("out_t", [OS, T], F32, kind="ExternalOutput")

    x8r = xT8.rearrange("(k p) n -> p k n", p=128)
    xbr = xTb.rearrange("(k p) n -> p k n", p=128)

    with tile.TileContext(nc, num_cores=NC) as tc:
        with (
            tc.tile_pool(name="const", bufs=1) as const,
            tc.tile_pool(name="attp", bufs=8) as attp,
            tc.tile_pool(name="xbpool", bufs=12) as xbpool,
            tc.tile_pool(name="ypool", bufs=6) as ypool,
            tc.tile_pool(name="ygpool", bufs=10) as ygpool,
            tc.tile_pool(name="outp", bufs=4) as outp,
            tc.tile_pool(name="sc_ps", bufs=3, space="PSUM") as sc_ps,
            tc.tile_pool(name="y_ps", bufs=2, space="PSUM") as y_ps,
            tc.tile_pool(name="dram", bufs=1, space="DRAM") as dram,
        ):
            # ---------------- prologue: weights / x / scalars ----------
            sq_sb = const.tile([128, KC, OS], F8, tag="sq")
            sk_sb = const.tile([128, KC, OS], F8, tag="sk")
            sq8r = sq8.rearrange("(k p) o -> p k o", p=128)
            sk8r = sk8.rearrange("(k p) o -> p k o", p=128)
            for k4 in range(0, KC, 4):
                nc.sync.dma_start(sq_sb[:, k4:k4 + 4, :], sq8r[:, k4:k4 + 4, :])
                nc.sync.dma_start(sk_sb[:, k4:k4 + 4, :], sk8r[:, k4:k4 + 4, :])
            scl_sb = const.tile([128, NHP, 6], F32, tag="scl")
            nc.sync.dma_start(scl_sb[:], scl_d.rearrange("(c p) o -> p c o", p=128))
            aq_sb = scl_sb[:, :, 0:1]
            ak_sb = scl_sb[:, :, 1:2]
            bq_sb = scl_sb[:, :, 2:3]
            bk_sb = scl_sb[:, :, 3:4]
            ap_sb = scl_sb[:, :, 4:5]
            bp_sb = scl_sb[:, :, 5:6]
            svv_sb = const.tile([DH, 2 * HPC], F32, tag="svv")
            nc.sync.dma_start(svv_sb[:], svv_d[:])
            av_sb = svv_sb[:, 0:HPC]
            bv_sb = svv_sb[:, HPC:2 * HPC]
            x8_sb = const.tile([128, KC, T], F8, tag="x8")
            for k2 in range(0, KC, 2):
                nc.sync.dma_start(x8_sb[:, k2:k2 + 2, :], x8r[:, k2:k2 + 2, :])
            sv_sb = const.tile([128, KC, OS], BF16, tag="sv")
            svbr = svb.rearrange("(k p) o -> p k o", p=128)
            for k4 in range(0, KC, 4):
                nc.sync.dma_start(sv_sb[:, k4:k4 + 4, :], svbr[:, k4:k4 + 4, :])
            # xb streamed per s-chunk (v matmul stationary): [128, KC, 128]
            sp_sb = const.tile([128, KC, OS], BF16, tag="sp")
            spbr = spb.rearrange("(k p) o -> p k o", p=128)
            for k4 in range(0, KC, 4):
                nc.sync.dma_start(sp_sb[:, k4:k4 + 4, :], spbr[:, k4:k4 + 4, :])

            # q,k per head-pair in bf16 [128 dims, T]; v in fp8
            # [s-part, scp, pair, head, DH+1] with a ones column for denoms.
            q_sb = const.tile([128, NHP, T], BF16, tag="qsb")
            k_sb = const.tile([128, NHP, T], BF16, tag="ksb")
            # inner dim padded to 66 so the DoubleRow pair step (8*66=528B)
            # meets the dual-fp8 ldweights 16B stride alignment
            v_sb = const.tile([128, NSC // 2, 2, HPC, DH + 2], F8, tag="vsb")
            nc.vector.memset(v_sb[:, :, :, :, DH:DH + 1], 1.0)

            y_gath = {}
            for tt in range(T // NT):
                for half in range(2):
                    yb = dram.tile([OS // 2, NT], BF16, tag=f"ybnc{tt}{half}")
                    yg = dram.tile([C // 2, NT], BF16, tag=f"ygth{tt}{half}")
                    y_gath[(tt, half)] = (yb, yg)

            # ---------------- QKV ---------------------------------------
            def qk_chunk(wn, hp, nt):
                s_sb, a_sb, b_sb, dst = {
                    "q": (sq_sb, aq_sb, bq_sb, q_sb),
                    "k": (sk_sb, ak_sb, bk_sb, k_sb),
                }[wn]
                psw = sc_ps.tile([128, 2, NT], F32, name=f"ps{wn}{hp}{nt}", tag="sps")
                ps = psw[:, 0, :]
                for j in range(KC // 2):
                    nc.tensor.matmul(
                        ps,
                        s_sb[:, 2 * j:2 * j + 2, hp * 128:(hp + 1) * 128],
                        x8_sb[:, 2 * j:2 * j + 2, nt * NT:(nt + 1) * NT],
                        start=(j == 0), stop=(j == KC // 2 - 1),
                        perf_mode=DR,
                    )
                nc.vector.tensor_scalar(
                    out=dst[:, hp, nt * NT:(nt + 1) * NT], in0=ps,
                    scalar1=a_sb[:, hp, :], scalar2=b_sb[:, hp, :],
                    op0=mybir.AluOpType.mult, op1=mybir.AluOpType.add,
                )

            def v_xload(sc):
                xbt = xbpool.tile([128, KC, 128], BF16, name=f"xb{sc}", tag="xbs")
                nc.sync.dma_start(xbt[:], xbr[:, :, sc * 128:(sc + 1) * 128])
                return xbt

            def v_chunk(sc, xbt):
                # swapped: stationary x bf16 chunk [128c, 128s],
                # moving sign cols [128c, 512 dims] -> psum [128 s, 512 d]
                psw = sc_ps.tile([128, 2, NT], F32, name=f"psv{sc}", tag="sps")
                ps = psw[:, 0, :]
                for kc in range(KC):
                    nc.tensor.matmul(
                        ps,
                        xbt[:, kc, :],
                        sv_sb[:, kc, :],
                        start=(kc == 0), stop=(kc == KC - 1),
                    )
                nc.vector.tensor_copy(
                    out=v_sb[:, sc // 2, sc % 2, :, 0:DH],
                    in_=ps.rearrange("p (h d) -> p h d", h=HPC),
                )

            # ---------------- attention ---------------------------------
            pend_norm = []

            def norm_stage1(hp, tt, h, yc):
                # issue the denominator reciprocal round-trips early so the
                # DRAM latency overlaps the next unit's attention
                hg = hp * 2 + h
                r_d = dram.tile([1, NT], F32, tag=f"rd{tt}{hg}")
                nc.sync.dma_start(r_d[:], yc[DH:DH + 1, :])
                rf = ypool.tile([DH, NT // DH], F32, tag="rf")
                nc.sync.dma_start(
                    rf[:], r_d.rearrange("one (p f) -> (one p) f", p=DH))
                rfi = ypool.tile([DH, NT // DH], F32, tag="rfi")
                nc.vector.reciprocal(rfi[:], rf[:])
                ri_d = dram.tile([DH, NT // DH], F32, tag=f"rid{tt}{hg}")
                nc.sync.dma_start(ri_d[:], rfi[:])
                rbi = ypool.tile([DH, NT], F32, tag="rbi")
                nc.sync.dma_start(
                    rbi[:],
                    bass.AP(tensor=ri_d.tensor, offset=ri_d.offset,
                            ap=[[0, DH], [1, NT]]),
                )
                return rbi

            def norm_stage2(item):
                hp, tt, h, yc, rbi = item
                hg = hp * 2 + h
                yt = ypool.tile([DH, NT], F32, tag="yt")
                nc.gpsimd.tensor_mul(yt[:], yc[0:DH, :], rbi[:])
                yb_out = ypool.tile([DH, NT], BF16, tag="ybf")
                nc.gpsimd.tensor_scalar(
                    out=yb_out[:], in0=yt[:],
                    scalar1=av_sb[:, hg:hg + 1], scalar2=bv_sb[:, hg:hg + 1],
                    op0=mybir.AluOpType.mult, op1=mybir.AluOpType.add,
                )
                half, row = divmod(hg * DH, OS // 2)
                nc.sync.dma_start(
                    y_gath[(tt, half)][0][row:row + DH, :], yb_out[:])

            def attention_unit(hp, tt, fills=()):
                fills = list(fills)
                t0 = tt * NT
                att_tiles = []
                for scp in range(NSC // 2):
                    at = attp.tile([128, 2, 2, NT], F8,
                                   name=f"at{hp}{tt}{scp}", tag="att")
                    att_tiles.append(at)
                for sc in range(NSC):
                    s0 = sc * 128
                    pss = sc_ps.tile([128, 2, NT], F32,
                                     name=f"s{hp}{tt}{sc}", tag="sps")
                    nc.tensor.matmul(
                        pss[:, 0, :], k_sb[0:DH, hp, s0:s0 + 128],
                        q_sb[0:DH, hp, t0:t0 + NT], start=True, stop=True,
                    )
                    nc.tensor.matmul(
                        pss[:, 1, :], k_sb[DH:128, hp, s0:s0 + 128],
                        q_sb[DH:128, hp, t0:t0 + NT], start=True, stop=True,
                    )
                    at = att_tiles[sc // 2]
                    if sc % DVE_EVERY == DVE_EVERY - 1:
                        nc.vector.tensor_scalar(
                            out=at[:, sc % 2, :, :].bitcast(I8), in0=pss[:],
                            scalar1=A8, scalar2=B8,
                            op0=mybir.AluOpType.mult, op1=mybir.AluOpType.add,
                        )
                    else:
                        nc.scalar.activation(
                            out=at[:, sc % 2, :, :], in_=pss[:],
                            func=mybir.ActivationFunctionType.Exp, scale=SCALE,
                        )
                    if sc % 4 == 3 and fills:
                        fills.pop(0)()
                while fills:
                    fills.pop(0)()
                psA = y_ps.tile([DH + 1, NT], F32, name=f"yA{hp}{tt}", tag="yps")
                psB = y_ps.tile([DH + 1, NT], F32, name=f"yB{hp}{tt}", tag="yps")
                for scp in range(NSC // 2):
                    at = att_tiles[scp]
                    for h, psy in ((0, psA), (1, psB)):
                        nc.tensor.matmul(
                            psy[:],
                            v_sb[:, scp, :, hp * 2 + h, 0:DH + 1],
                            at[:, :, h, :],
                            start=(scp == 0), stop=(scp == NSC // 2 - 1),
                            perf_mode=DR,
                        )
                for h, psy in ((0, psA), (1, psB)):
                    yc = ypool.tile([DH + 1, NT], F32,
                                    name=f"yc{hp}{tt}{h}", tag="yc")
                    nc.vector.tensor_copy(yc[:], psy[:])
                    rbi = norm_stage1(hp, tt, h, yc)
                    pend_norm.append((hp, tt, h, yc, rbi))
                while len(pend_norm) > 2:
                    norm_stage2(pend_norm.pop(0))

            def flush_norms():
                while pend_norm:
                    norm_stage2(pend_norm.pop(0))

            def gather_half(tt, half):
                yb, yg = y_gath[(tt, half)]
                nc.gpsimd.collective_compute(
                    "AllGather", mybir.AluOpType.bypass,
                    replica_groups=[[0, 1], [2, 3], [4, 5], [6, 7]],
                    ins=[yb.opt()], outs=[yg.opt()],
                )

            def _yg_load(tt, g):
                # gathered half h holds full-y rows [0:256]+[512:768] (h=0)
                # or [256:512]+[768:1024] (h=1)
                half, row = divmod((g % 4) * 128, OS // 2)
                src_t = y_gath[(tt, half)][1]
                row = row + (g // 4) * (OS // 2)
                yg_sb = ygpool.tile([128, NT], BF16, name=f"yg{tt}{g}", tag="ygp")
                nc.gpsimd.dma_start(yg_sb[:], src_t[row:row + 128, :])
                return yg_sb

            # g-chunks living in gather-half 0 vs half 1; chains consume the
            # half-1 chunks first (half-1 gathers mid-tt, half-0 at tt end)
            G_H0, G_H1 = (0, 1, 4, 5), (2, 3, 6, 7)
            G_ORDER = list(G_H1) + list(G_H0)

            def proj_oc(tt, oc, ygs, pps=None):
                for _ in (0,):
                    if pps is not None and oc in pps:
                        pp = pps[oc]
                        gseq = list(G_H0)
                        cont = True
                    else:
                        ppw = sc_ps.tile([128, 2, NT], F32,
                                         name=f"pp{tt}{oc}", tag="sps")
                        pp = ppw[:, 0, :]
                        gseq = list(G_ORDER)
                        cont = False
                    for i, g in enumerate(gseq):
                        nc.tensor.matmul(
                            pp, sp_sb[:, g, oc * 128:(oc + 1) * 128],
                            ygs[g][:], start=(not cont and i == 0),
                            stop=(i == len(gseq) - 1),
                        )
                    o_sb = outp.tile([128, NT], F32, name=f"o{tt}{oc}", tag="osb")
                    nc.vector.tensor_scalar(
                        out=o_sb[:], in0=pp,
                        scalar1=ap_sb[:, oc, :], scalar2=bp_sb[:, oc, :],
                        op0=mybir.AluOpType.mult, op1=mybir.AluOpType.add,
                    )
                    for pq in range(4):
                        nc.sync.dma_start(
                            out_t[oc * 128 + pq * 32:oc * 128 + (pq + 1) * 32,
                                  tt * NT:(tt + 1) * NT],
                            o_sb[pq * 32:(pq + 1) * 32, :])

            def proj_fills(tt, pps=None):
                ygs = {g: _yg_load(tt, g) for g in range(KC)}
                return [
                    (lambda oc=oc: proj_oc(tt, oc, ygs, pps))
                    for oc in range(NHP)
                ]

            def proj_first_half_fills(tt, pps):
                # fills that accumulate half-1 g-chunks for the first two
                # out-chunks in persistent psum tiles; the epilogue finishes
                # them with the half-0 chunks once the final gather lands
                ygs = {g: _yg_load(tt, g) for g in G_H1}
                pps["ygs_pre"] = ygs

                def one(oc):
                    ppw = sc_ps.tile([128, 2, NT], F32,
                                     name=f"pph{tt}{oc}", tag="sps")
                    pp = ppw[:, 0, :]
                    pps[oc] = pp
                    for i, g in enumerate(G_H1):
                        nc.tensor.matmul(
                            pp, sp_sb[:, g, oc * 128:(oc + 1) * 128],
                            ygs[g][:], start=(i == 0), stop=False,
                        )
                return [lambda oc=oc: one(oc) for oc in range(2)]

            # ---------------- schedule ----------------------------------
            for hp in range(NHP):
                for nt in range(T // NT):
                    qk_chunk("k", hp, nt)
            for hp in range(NHP):
                qk_chunk("q", hp, 0)
            v_xts = [v_xload(sc) for sc in range(NSC)]
            for sc in range(NSC // 2):
                v_chunk(sc, v_xts[sc])
            # second half of v and the remaining q chunks are emitted as
            # fills inside tt0's units so their PE work overlaps tt0's exp
            pend_fills = [
                (lambda sc=sc: v_chunk(sc, v_xts[sc]))
                for sc in range(NSC // 2, NSC)
            ] + [
                (lambda hp=hp, nt=nt: qk_chunk("q", hp, nt))
                for nt in range(1, T // NT) for hp in range(NHP)
            ]

            # the lag-2 pend_norm queue means: by the end of unit(hp, tt),
            # all norms of units two back are emitted -- so gather halves can
            # fire without explicit flushes (except the very last one).
            LT = T // NT - 1
            pps_last = {}
            HP_ORDER = (2, 3, 0, 1)
            for tt in range(T // NT):
                for ui, hp in enumerate(HP_ORDER):
                    if tt == 0:
                        nfill = 8 if ui == 0 else 4
                    elif tt == LT and ui == 3:
                        nfill = 4
                    else:
                        nfill = 2
                    take, pend_fills = pend_fills[:nfill], pend_fills[nfill:]
                    if tt == LT and ui == 3:
                        take = take + proj_first_half_fills(LT, pps_last)
                    attention_unit(hp, tt, take)
                    if ui == 2 and tt >= 1:
                        pend_fills = pend_fills + proj_fills(tt - 1)
                    if ui == 2:
                        gather_half(tt, 1)
                flush_norms()
                gather_half(tt, 0)
            for f in pend_fills:
                f()
            ygs_all = pps_last.pop("ygs_pre")
            ygs_all.update({g: _yg_load(LT, g) for g in G_H0})
            for oc in (2, 3, 0, 1):
                proj_oc(LT, oc, ygs_all, pps_last)

    nc.finalize()
    return nc


def _host_prep(x, Wq, bq, Wk, bk, Wv, bv, Wp, bp):
    F8N = ml_dtypes.float8_e4m3
    BF = ml_dtypes.bfloat16
    in_maps = []
    xt_b = [np.ascontiguousarray(x[b].T) for b in range(B)]
    for c in range(NC):
        b, j = c // 2, c % 2
        hs = slice(OS * j, OS * (j + 1))
        m = {
            "xT8": xt_b[b].astype(F8N),
            "xTb": xt_b[b].astype(BF),
            "sq8": np.ascontiguousarray(np.sign(Wq[hs]).T).astype(F8N),
            "sk8": np.ascontiguousarray(np.sign(Wk[hs]).T).astype(F8N),
            "svb": np.ascontiguousarray(np.sign(Wv[hs]).T).astype(BF),
            "spb": np.ascontiguousarray(np.sign(Wp[hs]).T).astype(BF),
            "scl": np.stack([
                np.abs(Wq[hs]).mean(1, dtype=np.float64).astype(np.float32),
                np.abs(Wk[hs]).mean(1, dtype=np.float64).astype(np.float32),
                bq[hs], bk[hs],
                np.abs(Wp[hs]).mean(1, dtype=np.float64).astype(np.float32),
                bp[hs]], axis=1),
            "svv": np.concatenate([
                np.abs(Wv[hs]).mean(1, dtype=np.float64)
                .reshape(HPC, DH).T.astype(np.float32),
                bv[hs].reshape(HPC, DH).T.astype(np.float32)], axis=1),
        }
        in_maps.append(m)
    return in_maps


def kernel(x, Wq, bq, Wk, bk, Wv, bv, Wp, bp, _trace=False, _trace_cores=None):
    if "nc" not in _CACHED:
        _CACHED["nc"] = _build()
    nc = _CACHED["nc"]
    in_maps = _host_prep(x, Wq, bq, Wk, bk, Wv, bv, Wp, bp)
    res = run_bass_kernel_spmd(
        nc, in_maps, core_ids=list(range(NC)),
        trace=_trace, trace_cores=_trace_cores,
    )
    _CACHED["last_results"] = res
    out = np.empty((B, T, C), dtype=np.float32)
    for b in range(B):
        full = np.concatenate(
            [res.results[2 * b]["out_t"], res.results[2 * b + 1]["out_t"]],
            axis=0)                     # [1024 o, 2048 t]
        out[b] = full.T
    return out


# revision 36
# speedup vs baseline: 1.1563x; 1.1502x over previous
"""BinaryAttention on 8 TRN2 NeuronCores (Bass/Tile, SPMD).

Math (per reference):
  Wb = alpha * sign(W), alpha[o] = mean_c |W[o,c]|
  q/k/v = x @ Wb_{q,k,v}^T + b;   att = softmax(q k^T / sqrt(Dh));
  y = att @ v;  out = y @ Wb_p^T + bp

Sharding (8 cores = 4 batch groups x 2 cores): core c handles batch c//2
with heads [8j, 8j+8) for j = c%2 (head-tensor-parallel within the pair).
After attention, a pairwise AllGather assembles y [1024, T_tile] per pair;
proj is output-column sharded (core j computes out cols [512j, 512j+512)).

Precision plan (validated vs reference in fp64/numpy, rel ~1.4e-2 < 2e-2):
  - q,k matmuls: fp8(e4m3) x and sign-weights, DoubleRow perf mode (2x);
    alpha/bias applied fp32 -> q,k in bf16.
  - scores: bf16, two PE row-tiles (heads at partitions 0-63 / 64-127).
  - exp: Scalar engine exact exp -> fp8 att for 3/4 of s-chunks; DVE
    computes a bit-trick fast exp (int8 = s*A + B bitcast as e4m3) for 1/4.
  - att@v: fp8 DoubleRow (2x); v kept unscaled (alpha_v/bias_v folded into
    the normalization: y = alpha_v*(ym/den) + bias_v).
  - v matmul: bf16 "swapped" form (stationary x-chunks, moving sign-cols)
    which yields v in [s, dims] layout directly -- no PE transposes.
  - proj: bf16 (fp8 y would push error past tolerance).
"""

import numpy as np
import ml_dtypes

import concourse.bass as bass
import concourse.bacc as bacc
import concourse.tile as tile
from concourse import mybir
from concourse.bass_utils import run_bass_kernel_spmd

NC = 8
B, T, C = 4, 2048, 1024
H, DH = 16, 64
HPC = 8          # heads per core
NHP = 4          # head-pairs per core
OS = 512         # per-core o-slice (8 heads * 64 = 512 dims)
KC = C // 128    # contraction chunks
NT = 512         # t-tile (one psum bank of fp32)
NSC = T // 128   # s-chunks (16)
SCALE = DH ** -0.5
LOG2E = 1.4426950408889634
# DVE fast-exp: e4m3 bits = round(s*scale*log2e*8 + 56 + C8)
A8 = SCALE * LOG2E * 8.0
B8 = 56.0 - 0.5
DVE_EVERY = 3    # every 3rd s-chunk's exp goes to DVE

F32 = mybir.dt.float32
BF16 = mybir.dt.bfloat16
F8 = mybir.dt.float8e4
I8 = mybir.dt.int8
DR = mybir.MatmulPerfMode.DoubleRow

_CACHED = {}


def _build():
    nc = bacc.Bacc("TRN2", target_bir_lowering=False, debug=False, num_devices=NC)

    xT8 = nc.dram_tensor("xT8", [C, T], F8, kind="ExternalInput")
    xTb = nc.dram_tensor("xTb", [C, T], BF16, kind="ExternalInput")
    sq8 = nc.dram_tensor("sq8", [C, OS], F8, kind="ExternalInput")
    sk8 = nc.dram_tensor("sk8", [C, OS], F8, kind="ExternalInput")
    svb = nc.dram_tensor("svb", [C, OS], BF16, kind="ExternalInput")
    spb = nc.dram_tensor("spb", [C, OS], BF16, kind="ExternalInput")
    scl_d = nc.dram_tensor("scl", [OS, 6], F32, kind="ExternalInput")
    svv_d = nc.dram_tensor("svv", [DH, 2 * HPC], F32, kind="ExternalInput")
    out_t = nc.dram_tensor("out_t", [OS, T], F32, kind="ExternalOutput")

    x8r = xT8.rearrange("(k p) n -> p k n", p=128)
    xbr = xTb.rearrange("(k p) n -> p k n", p=128)

    with tile.TileContext(nc, num_cores=NC) as tc:
        with (
            tc.tile_pool(name="const", bufs=1) as const,
            tc.tile_pool(name="attp", bufs=8) as attp,
            tc.tile_pool(name="xbpool", bufs=12) as xbpool,
            tc.tile_pool(name="ypool", bufs=6) as ypool,
            tc.tile_pool(name="ygpool", bufs=10) as ygpool,
            tc.tile_pool(name="outp", bufs=4) as outp,
            tc.tile_pool(name="sc_ps", bufs=3, space="PSUM") as sc_ps,
            tc.tile_pool(name="y_ps", bufs=2, space="PSUM") as y_ps,
            tc.tile_pool(name="dram", bufs=1, space="DRAM") as dram,
        ):
            # ---------------- prologue: weights / x / scalars ----------
            sq_sb = const.tile([128, KC, OS], F8, tag="sq")
            sk_sb = const.tile([128, KC, OS], F8, tag="sk")
            sq8r = sq8.rearrange("(k p) o -> p k o", p=128)
            sk8r = sk8.rearrange("(k p) o -> p k o", p=128)
            for k4 in range(0, KC, 4):
                nc.sync.dma_start(sq_sb[:, k4:k4 + 4, :], sq8r[:, k4:k4 + 4, :])
                nc.sync.dma_start(sk_sb[:, k4:k4 + 4, :], sk8r[:, k4:k4 + 4, :])
            scl_sb = const.tile([128, NHP, 6], F32, tag="scl")
            nc.sync.dma_start(scl_sb[:], scl_d.rearrange("(c p) o -> p c o", p=128))
            aq_sb = scl_sb[:, :, 0:1]
            ak_sb = scl_sb[:, :, 1:2]
            bq_sb = scl_sb[:, :, 2:3]
            bk_sb = scl_sb[:, :, 3:4]
            ap_sb = scl_sb[:, :, 4:5]
            bp_sb = scl_sb[:, :, 5:6]
            svv_sb = const.tile([DH, 2 * HPC], F32, tag="svv")
            nc.sync.dma_start(svv_sb[:], svv_d[:])
            av_sb = svv_sb[:, 0:HPC]
            bv_sb = svv_sb[:, HPC:2 * HPC]
            x8_sb = const.tile([128, KC, T], F8, tag="x8")
            for k2 in range(0, KC, 2):
                nc.sync.dma_start(x8_sb[:, k2:k2 + 2, :], x8r[:, k2:k2 + 2, :])
            sv_sb = const.tile([128, KC, OS], BF16, tag="sv")
            svbr = svb.rearrange("(k p) o -> p k o", p=128)
            for k4 in range(0, KC, 4):
                nc.sync.dma_start(sv_sb[:, k4:k4 + 4, :], svbr[:, k4:k4 + 4, :])
            # xb streamed per s-chunk (v matmul stationary): [128, KC, 128]
            sp_sb = const.tile([128, KC, OS], BF16, tag="sp")
            spbr = spb.rearrange("(k p) o -> p k o", p=128)
            for k4 in range(0, KC, 4):
                nc.sync.dma_start(sp_sb[:, k4:k4 + 4, :], spbr[:, k4:k4 + 4, :])

            # q,k per head-pair in bf16 [128 dims, T]; v in fp8
            # [s-part, scp, pair, head, DH+1] with a ones column for denoms.
            q_sb = const.tile([128, NHP, T], BF16, tag="qsb")
            k_sb = const.tile([128, NHP, T], BF16, tag="ksb")
            # inner dim padded to 66 so the DoubleRow pair step (8*66=528B)
            # meets the dual-fp8 ldweights 16B stride alignment
            v_sb = const.tile([128, NSC // 2, 2, HPC, DH + 2], F8, tag="vsb")
            nc.vector.memset(v_sb[:, :, :, :, DH:DH + 1], 1.0)

            y_gath = {}
            LT = T // NT - 1
            for tt in range(T // NT):
                for half in range(2):
                    if tt == LT and half == 0:
                        continue
                    yb = dram.tile([OS // 2, NT], BF16, tag=f"ybnc{tt}{half}")
                    yg = dram.tile([C // 2, NT], BF16, tag=f"ygth{tt}{half}")
                    y_gath[(tt, half)] = (yb, yg)
            for q in ("0a", "0b"):
                yb = dram.tile([OS // 4, NT], BF16, tag=f"ybnc{LT}{q}")
                yg = dram.tile([C // 4, NT], BF16, tag=f"ygth{LT}{q}")
                y_gath[(LT, q)] = (yb, yg)

            # ---------------- QKV ---------------------------------------
            def qk_chunk(wn, hp, nt):
                s_sb, a_sb, b_sb, dst = {
                    "q": (sq_sb, aq_sb, bq_sb, q_sb),
                    "k": (sk_sb, ak_sb, bk_sb, k_sb),
                }[wn]
                psw = sc_ps.tile([128, 2, NT], F32, name=f"ps{wn}{hp}{nt}", tag="sps")
                ps = psw[:, 0, :]
                for j in range(KC // 2):
                    nc.tensor.matmul(
                        ps,
                        s_sb[:, 2 * j:2 * j + 2, hp * 128:(hp + 1) * 128],
                        x8_sb[:, 2 * j:2 * j + 2, nt * NT:(nt + 1) * NT],
                        start=(j == 0), stop=(j == KC // 2 - 1),
                        perf_mode=DR,
                    )
                nc.vector.tensor_scalar(
                    out=dst[:, hp, nt * NT:(nt + 1) * NT], in0=ps,
                    scalar1=a_sb[:, hp, :], scalar2=b_sb[:, hp, :],
                    op0=mybir.AluOpType.mult, op1=mybir.AluOpType.add,
                )

            def v_xload(sc):
                xbt = xbpool.tile([128, KC, 128], BF16, name=f"xb{sc}", tag="xbs")
                nc.sync.dma_start(xbt[:], xbr[:, :, sc * 128:(sc + 1) * 128])
                return xbt

            def v_chunk(sc, xbt):
                # swapped: stationary x bf16 chunk [128c, 128s],
                # moving sign cols [128c, 512 dims] -> psum [128 s, 512 d]
                psw = sc_ps.tile([128, 2, NT], F32, name=f"psv{sc}", tag="sps")
                ps = psw[:, 0, :]
                for kc in range(KC):
                    nc.tensor.matmul(
                        ps,
                        xbt[:, kc, :],
                        sv_sb[:, kc, :],
                        start=(kc == 0), stop=(kc == KC - 1),
                    )
                nc.vector.tensor_copy(
                    out=v_sb[:, sc // 2, sc % 2, :, 0:DH],
                    in_=ps.rearrange("p (h d) -> p h d", h=HPC),
                )

            # ---------------- attention ---------------------------------
            pend_norm = []

            def norm_stage1(hp, tt, h, yc):
                # issue the denominator reciprocal round-trips early so the
                # DRAM latency overlaps the next unit's attention
                hg = hp * 2 + h
                r_d = dram.tile([1, NT], F32, tag=f"rd{tt}{hg}")
                nc.sync.dma_start(r_d[:], yc[DH:DH + 1, :])
                rf = ypool.tile([DH, NT // DH], F32, tag="rf")
                nc.sync.dma_start(
                    rf[:], r_d.rearrange("one (p f) -> (one p) f", p=DH))
                rfi = ypool.tile([DH, NT // DH], F32, tag="rfi")
                nc.vector.reciprocal(rfi[:], rf[:])
                ri_d = dram.tile([DH, NT // DH], F32, tag=f"rid{tt}{hg}")
                nc.sync.dma_start(ri_d[:], rfi[:])
                rbi = ypool.tile([DH, NT], F32, tag="rbi")
                nc.sync.dma_start(
                    rbi[:],
                    bass.AP(tensor=ri_d.tensor, offset=ri_d.offset,
                            ap=[[0, DH], [1, NT]]),
                )
                return rbi

            def norm_stage2(item):
                hp, tt, h, yc, rbi = item
                hg = hp * 2 + h
                yt = ypool.tile([DH, NT], F32, tag="yt")
                nc.gpsimd.tensor_mul(yt[:], yc[0:DH, :], rbi[:])
                yb_out = ypool.tile([DH, NT], BF16, tag="ybf")
                nc.gpsimd.tensor_scalar(
                    out=yb_out[:], in0=yt[:],
                    scalar1=av_sb[:, hg:hg + 1], scalar2=bv_sb[:, hg:hg + 1],
                    op0=mybir.AluOpType.mult, op1=mybir.AluOpType.add,
                )
                half, row = divmod(hg * DH, OS // 2)
                if tt == LT and half == 0:
                    half = "0a" if row < OS // 4 else "0b"
                    row = row % (OS // 4)
                nc.sync.dma_start(
                    y_gath[(tt, half)][0][row:row + DH, :], yb_out[:])

            def attention_unit(hp, tt, fills=()):
                fills = list(fills)
                t0 = tt * NT
                att_tiles = []
                for scp in range(NSC // 2):
                    at = attp.tile([128, 2, 2, NT], F8,
                                   name=f"at{hp}{tt}{scp}", tag="att")
                    att_tiles.append(at)
                for sc in range(NSC):
                    s0 = sc * 128
                    pss = sc_ps.tile([128, 2, NT], F32,
                                     name=f"s{hp}{tt}{sc}", tag="sps")
                    nc.tensor.matmul(
                        pss[:, 0, :], k_sb[0:DH, hp, s0:s0 + 128],
                        q_sb[0:DH, hp, t0:t0 + NT], start=True, stop=True,
                    )
                    nc.tensor.matmul(
                        pss[:, 1, :], k_sb[DH:128, hp, s0:s0 + 128],
                        q_sb[DH:128, hp, t0:t0 + NT], start=True, stop=True,
                    )
                    at = att_tiles[sc // 2]
                    if sc % DVE_EVERY == DVE_EVERY - 1:
                        nc.vector.tensor_scalar(
                            out=at[:, sc % 2, :, :].bitcast(I8), in0=pss[:],
                            scalar1=A8, scalar2=B8,
                            op0=mybir.AluOpType.mult, op1=mybir.AluOpType.add,
                        )
                    else:
                        nc.scalar.activation(
                            out=at[:, sc % 2, :, :], in_=pss[:],
                            func=mybir.ActivationFunctionType.Exp, scale=SCALE,
                        )
                    if sc % 4 == 3 and fills:
                        fills.pop(0)()
                while fills:
                    fills.pop(0)()
                psA = y_ps.tile([DH + 1, NT], F32, name=f"yA{hp}{tt}", tag="yps")
                psB = y_ps.tile([DH + 1, NT], F32, name=f"yB{hp}{tt}", tag="yps")
                for scp in range(NSC // 2):
                    at = att_tiles[scp]
                    for h, psy in ((0, psA), (1, psB)):
                        nc.tensor.matmul(
                            psy[:],
                            v_sb[:, scp, :, hp * 2 + h, 0:DH + 1],
                            at[:, :, h, :],
                            start=(scp == 0), stop=(scp == NSC // 2 - 1),
                            perf_mode=DR,
                        )
                for h, psy in ((0, psA), (1, psB)):
                    yc = ypool.tile([DH + 1, NT], F32,
                                    name=f"yc{hp}{tt}{h}", tag="yc")
                    nc.vector.tensor_copy(yc[:], psy[:])
                    rbi = norm_stage1(hp, tt, h, yc)
                    pend_norm.append((hp, tt, h, yc, rbi))
                while len(pend_norm) > 2:
                    norm_stage2(pend_norm.pop(0))

            def flush_norms():
                while pend_norm:
                    norm_stage2(pend_norm.pop(0))

            def gather_half(tt, half):
                yb, yg = y_gath[(tt, half)]
                nc.gpsimd.collective_compute(
                    "AllGather", mybir.AluOpType.bypass,
                    replica_groups=[[0, 1], [2, 3], [4, 5], [6, 7]],
                    ins=[yb.opt()], outs=[yg.opt()],
                )

            def _yg_load(tt, g):
                # gathered half h holds full-y rows [0:256]+[512:768] (h=0)
                # or [256:512]+[768:1024] (h=1)
                half, row = divmod((g % 4) * 128, OS // 2)
                if tt == LT and half == 0:
                    half = "0a" if row == 0 else "0b"
                    row = (g // 4) * (OS // 4)
                else:
                    row = row + (g // 4) * (OS // 2)
                src_t = y_gath[(tt, half)][1]
                yg_sb = ygpool.tile([128, NT], BF16, name=f"yg{tt}{g}", tag="ygp")
                nc.gpsimd.dma_start(yg_sb[:], src_t[row:row + 128, :])
                return yg_sb

            # g-chunks living in gather-half 0 vs half 1; chains consume the
            # half-1 chunks first (half-1 gathers mid-tt, half-0 at tt end)
            G_H0, G_H1 = (0, 1, 4, 5), (2, 3, 6, 7)
            G_ORDER = list(G_H1) + list(G_H0)

            def proj_oc(tt, oc, ygs, pps=None):
                for _ in (0,):
                    if pps is not None and oc in pps:
                        pp = pps[oc]
                        gseq = [0, 4, 1, 5]
                        cont = True
                    else:
                        ppw = sc_ps.tile([128, 2, NT], F32,
                                         name=f"pp{tt}{oc}", tag="sps")
                        pp = ppw[:, 0, :]
                        gseq = list(G_ORDER)
                        cont = False
                    for i, g in enumerate(gseq):
                        nc.tensor.matmul(
                            pp, sp_sb[:, g, oc * 128:(oc + 1) * 128],
                            ygs[g][:], start=(not cont and i == 0),
                            stop=(i == len(gseq) - 1),
                        )
                    o_sb = outp.tile([128, NT], F32, name=f"o{tt}{oc}", tag="osb")
                    nc.vector.tensor_scalar(
                        out=o_sb[:], in0=pp,
                        scalar1=ap_sb[:, oc, :], scalar2=bp_sb[:, oc, :],
                        op0=mybir.AluOpType.mult, op1=mybir.AluOpType.add,
                    )
                    for pq in range(4):
                        nc.sync.dma_start(
                            out_t[oc * 128 + pq * 32:oc * 128 + (pq + 1) * 32,
                                  tt * NT:(tt + 1) * NT],
                            o_sb[pq * 32:(pq + 1) * 32, :])

            def proj_fills(tt, pps=None):
                ygs = {g: _yg_load(tt, g) for g in range(KC)}
                return [
                    (lambda oc=oc: proj_oc(tt, oc, ygs, pps))
                    for oc in range(NHP)
                ]

            def proj_first_half_fills(tt, pps):
                # fills that accumulate half-1 g-chunks for the first two
                # out-chunks in persistent psum tiles; the epilogue finishes
                # them with the half-0 chunks once the final gather lands
                ygs = {g: _yg_load(tt, g) for g in G_H1}
                pps["ygs_pre"] = ygs

                def one(oc):
                    ppw = sc_ps.tile([128, 2, NT], F32,
                                     name=f"pph{tt}{oc}", tag="sps")
                    pp = ppw[:, 0, :]
                    pps[oc] = pp
                    for i, g in enumerate(G_H1):
                        nc.tensor.matmul(
                            pp, sp_sb[:, g, oc * 128:(oc + 1) * 128],
                            ygs[g][:], start=(i == 0), stop=False,
                        )
                return [lambda oc=oc: one(oc) for oc in range(2)]

            # ---------------- schedule ----------------------------------
            for hp in range(NHP):
                for nt in range(T // NT):
                    qk_chunk("k", hp, nt)
            for hp in range(NHP):
                qk_chunk("q", hp, 0)
            v_xts = [v_xload(sc) for sc in range(NSC)]
            for sc in range(NSC // 2):
                v_chunk(sc, v_xts[sc])
            # second half of v and the remaining q chunks are emitted as
            # fills inside tt0's units so their PE work overlaps tt0's exp
            pend_fills = [
                (lambda sc=sc: v_chunk(sc, v_xts[sc]))
                for sc in range(NSC // 2, NSC)
            ] + [
                (lambda hp=hp, nt=nt: qk_chunk("q", hp, nt))
                for nt in range(1, T // NT) for hp in range(NHP)
            ]

            # the lag-2 pend_norm queue means: by the end of unit(hp, tt),
            # all norms of units two back are emitted -- so gather halves can
            # fire without explicit flushes (except the very last one).
            pps_last = {}
            HP_ORDER = (2, 3, 0, 1)
            for tt in range(T // NT):
                for ui, hp in enumerate(HP_ORDER):
                    if tt == 0:
                        nfill = 8 if ui == 0 else 4
                    elif tt == LT and ui == 3:
                        nfill = 4
                    else:
                        nfill = 2
                    take, pend_fills = pend_fills[:nfill], pend_fills[nfill:]
                    if tt == LT and ui == 3:
                        take = take + proj_first_half_fills(LT, pps_last)
                    attention_unit(hp, tt, take)
                    if ui == 2 and tt >= 1:
                        pend_fills = pend_fills + proj_fills(tt - 1)
                    if ui == 2:
                        gather_half(tt, 1)
                    if tt == LT and ui == 3:
                        gather_half(LT, "0a")
                flush_norms()
                gather_half(tt, 0 if tt < LT else "0b")
            for f in pend_fills:
                f()
            ygs_all = pps_last.pop("ygs_pre")
            ygs_all.update({g: _yg_load(LT, g) for g in G_H0})
            for oc in (2, 3, 0, 1):
                proj_oc(LT, oc, ygs_all, pps_last)

    nc.finalize()
    return nc


def _host_prep(x, Wq, bq, Wk, bk, Wv, bv, Wp, bp):
    F8N = ml_dtypes.float8_e4m3
    BF = ml_dtypes.bfloat16
    in_maps = []
    xt_b = [np.ascontiguousarray(x[b].T) for b in range(B)]
    for c in range(NC):
        b, j = c // 2, c % 2
        hs = slice(OS * j, OS * (j + 1))
        m = {
            "xT8": xt_b[b].astype(F8N),
            "xTb": xt_b[b].astype(BF),
            "sq8": np.ascontiguousarray(np.sign(Wq[hs]).T).astype(F8N),
            "sk8": np.ascontiguousarray(np.sign(Wk[hs]).T).astype(F8N),
            "svb": np.ascontiguousarray(np.sign(Wv[hs]).T).astype(BF),
            "spb": np.ascontiguousarray(np.sign(Wp[hs]).T).astype(BF),
            "scl": np.stack([
                np.abs(Wq[hs]).mean(1, dtype=np.float64).astype(np.float32),
                np.abs(Wk[hs]).mean(1, dtype=np.float64).astype(np.float32),
                bq[hs], bk[hs],
                np.abs(Wp[hs]).mean(1, dtype=np.float64).astype(np.float32),
                bp[hs]], axis=1),
            "svv": np.concatenate([
                np.abs(Wv[hs]).mean(1, dtype=np.float64)
                .reshape(HPC, DH).T.astype(np.float32),
                bv[hs].reshape(HPC, DH).T.astype(np.float32)], axis=1),
        }
        in_maps.append(m)
    return in_maps


def kernel(x, Wq, bq, Wk, bk, Wv, bv, Wp, bp, _trace=False, _trace_cores=None):
    if "nc" not in _CACHED:
        _CACHED["nc"] = _build()
    nc = _CACHED["nc"]
    in_maps = _host_prep(x, Wq, bq, Wk, bk, Wv, bv, Wp, bp)
    res = run_bass_kernel_spmd(
        nc, in_maps, core_ids=list(range(NC)),
        trace=_trace, trace_cores=_trace_cores,
    )
    _CACHED["last_results"] = res
    out = np.empty((B, T, C), dtype=np.float32)
    for b in range(B):
        full = np.concatenate(
            [res.results[2 * b]["out_t"], res.results[2 * b + 1]["out_t"]],
            axis=0)                     # [1024 o, 2048 t]
        out[b] = full.T
    return out
